# revision 54
# baseline (speedup 1.0000x reference)
"""Trainium2 Bass kernel for nn_Interpolator: zero-stuff upsample x8 + 128-tap FIR (SAME) + x8 gain.

Polyphase formulation: with m indexing 64-sample rows of x and n = 8*q' + r in [0, 512),
    y[512*m + n] = sum_{k=0}^{78} T4[k, m] * H4[k, n]
where T4[k, m] = x[64*m + k - 7] (zero-padded) and
    H4[k, 8*q'+r] = 8 * h[(7-r) + 8*(k-q')]  for 0 <= k-q' <= 15, else 0.

Per core (8 cores, batch-parallel): 16 signals (8 batch rows x {real, imag}).

The im2col (T4) is built on the HOST and shipped as a [128, 512 + 16*512] fp16
DRAM tensor (k-major; rows 79-127 junk), so the kernel needs NO xbar
DMA-transposes at all — that removes the serial xbar block + mode-transition
drains of the v1 kernel.  Cols 0-511 carry H4 so the first chunked load
delivers the streamed filter matrix AND sig 0's first weights in ONE DMA (one
completion receipt on the critical path instead of two).  All loads and stores
are plain HWDGE DMAs on the sync queue; stores use a PERMUTED, per-partition
contiguous DRAM layout (y_dev[sig, i, 512*t + n] = y[sig, 65536*t + 512*i + n],
AP [[2048,128],[1,2048]] = 128 x 4KB descriptors); the host un-permutes.

The PE is clock-capped at 1.2 GHz on this part (the HAM never lifts K=4/8 even
after 27 us of gapless matmuls; under thermal pressure it sags to ~1.1), so
the 64 matmuls (512 streamed cols each, 427 ns) are the 27.3 us critical path;
loads/casts/stores pipeline behind it with zero MM gaps.  Each matmul gets its
OWN single-bank [128, 512] PSUM tile (8 bufs = all 8 banks = 2 signals of
pipeline depth): per-MM casts alternate vector/scalar, and no cast ever shares
a PSUM tile with a pending matmul (Tile serializes same-tile access at tile
granularity, which would stall the PE).  Store completions (the HBM
write-receipt that fires each DMA's final sem inc) process SERIALLY at ~320 ns
per store and ~1-3 us behind the data, so the kernel-end drain is gated by the
receipt chain of the last few stores: every signal stores in TWO 256 KB halves
(the first issued mid-compute) so receipts spread out instead of bunching, and
the last signal's final matmul is split N=384+N=128 (same stationary, zero
extra PE cycles) so the very last cast/store are tiny and the drain starts as
early as possible after the last PE cycle.  y is fp16 on device; the host
casts to fp32.
"""

import numpy as np
from numpy.lib.stride_tricks import sliding_window_view

import concourse.bass as bass
import concourse.tile as tile
from concourse import bacc, mybir
from concourse.bass_utils import run_bass_kernel_spmd

B = 64
N = 32768
FACTOR = 8
NOUT = N * FACTOR  # 262144
N_CORES = 8
ROWS_PER_CORE = B // N_CORES  # 8
SIGS = 2 * ROWS_PER_CORE  # 16 signals per core (real rows then imag rows)
K = 79  # contraction window length
NPAD = 32832  # 7 leading zeros + N + 57 trailing zeros; = 64*513
M = 512  # T4 columns (m-values) per signal
TILES = 4  # out tiles per signal, each [128 m-rows, 512 samples]

# Load chunks in units of 128 T4 columns (1 signal = 4 units); the first
# chunk carries H4 (512 lead cols) + sig 0's first matmul tile so the PE can
# start as early as possible.
CHUNK_UNITS = (2, 6, 12, 20, 24)  # sums to 64 = SIGS*4

_F16 = mybir.dt.float16
_F32 = mybir.dt.float32

_NC_CACHE = {}


def _build_nc():
    nc = bacc.Bacc(
        "TRN2",
        target_bir_lowering=False,
        debug=False,
        enable_asserts=False,
        num_devices=N_CORES,
    )
    # t4x holds [128 rows, 512 + SIGS*512 cols]: cols 0-511 = H4 (rows 79+ junk),
    # cols 512.. = T4 of the 16 signals (512 m-cols each)
    t4 = nc.dram_tensor("t4", [128 * (512 + SIGS * M)], _F16, kind="ExternalInput")
    y = nc.dram_tensor("y", [SIGS, NOUT], _F16, kind="ExternalOutput")

    with tile.TileContext(nc) as tc:
        with (
            tc.tile_pool(name="t4pool", bufs=len(CHUNK_UNITS)) as t4pool,
            tc.tile_pool(name="opool", bufs=8) as opool,
            tc.tile_pool(name="po", bufs=8, space="PSUM") as po_pool,
        ):
            # per-(sig, t) map to (tile, local col base) after its chunk's load
            t4_of_tile = {}
            h4_sb = None  # set by the first chunk load (h4 rides along)
            DSTRIDE = 512 + SIGS * M  # row stride of t4x in DRAM

            def load_chunk(first_unit, n_units):
                nonlocal h4_sb
                lead = 512 if first_unit == 0 else 0  # h4 rides in chunk 0
                w = 128 * n_units + lead
                T4g = t4pool.tile([128, w], _F16, tag="t4")
                off = 512 + first_unit * 128 - lead
                nc.sync.dma_start(
                    out=T4g[:, :],
                    in_=bass.AP(tensor=t4, offset=off, ap=[[DSTRIDE, 128], [1, w]]),
                )
                if first_unit == 0:
                    h4_sb = T4g
                for u in range(n_units):
                    g = first_unit + u
                    t4_of_tile[(g // 4, g % 4)] = (T4g, 128 * u + lead)

            def store_cols(sig, out_sb, c0, c1, eng=None):
                (eng or nc.sync).dma_start(
                    out=bass.AP(
                        tensor=y,
                        offset=sig * NOUT + c0,
                        ap=[[2048, 128], [1, c1 - c0]],
                    ),
                    in_=out_sb[:, c0:c1],
                )

            def mm(sig, t, po_slice):
                T4g, base = t4_of_tile[(sig, t)]
                nc.tensor.matmul(
                    po_slice,
                    T4g[0:K, base : base + 128],
                    h4_sb[0:K, 0:512],
                    start=True,
                    stop=True,
                )

            def compute_store(sig):
                """4 matmuls, each into its own single-bank PSUM tile, per-MM
                casts alternating engines, one 512 KB store."""
                out_sb = opool.tile([128, TILES * 512], _F16)
                for t in range(TILES):
                    po = po_pool.tile([128, 512], _F32, tag="po")
                    mm(sig, t, po[:, :])
                    sl = slice(512 * t, 512 * (t + 1))
                    if t % 2 == 0:
                        nc.vector.tensor_copy(out=out_sb[:, sl], in_=po[:, :])
                    else:
                        nc.scalar.copy(out=out_sb[:, sl], in_=po[:, :])
                    if t == 1:
                        # store the first half early: receipts spread out
                        # instead of bunching (they serialize at ~320ns), and
                        # the final drain only waits on the last small pieces
                        store_cols(sig, out_sb, 0, 1024)
                # store: y_dev[sig, i, c] = out_sb[i, c]  (per-partition 4KB contig)
                store_cols(sig, out_sb, 1024, 2048)

            def compute_store_last(sig):
                """Last signal: the final matmul is split N=384+N=128 (same
                stationary, zero extra PE cycles) so the very last cast and
                store are tiny and the end-of-kernel drain's receipt chain
                starts as early as possible after the last PE cycle."""
                out_sb = opool.tile([128, TILES * 512], _F16)
                po0 = po_pool.tile([128, 512], _F32, tag="po")
                mm(sig, 0, po0[:, :])
                nc.vector.tensor_copy(out=out_sb[:, 0:512], in_=po0[:, :])
                po1 = po_pool.tile([128, 512], _F32, tag="po")
                mm(sig, 1, po1[:, :])
                nc.scalar.copy(out=out_sb[:, 512:1024], in_=po1[:, :])
                store_cols(sig, out_sb, 0, 1024)  # streams while t2/t3 compute
                po2 = po_pool.tile([128, 512], _F32, tag="po")
                mm(sig, 2, po2[:, :])
                nc.vector.tensor_copy(out=out_sb[:, 1024:1536], in_=po2[:, :])
                T4g, base = t4_of_tile[(sig, 3)]
                po3a = po_pool.tile([128, 512], _F32, tag="po")
                nc.tensor.matmul(
                    po3a[:, 0:384],
                    T4g[0:K, base : base + 128],
                    h4_sb[0:K, 0:384],
                    start=True,
                    stop=True,
                )
                po3b = po_pool.tile([128, 512], _F32, tag="po")
                nc.tensor.matmul(
                    po3b[:, 0:128],
                    T4g[0:K, base : base + 128],
                    h4_sb[0:K, 384:512],
                    start=True,
                    stop=True,
                )
                nc.scalar.copy(out=out_sb[:, 1536:1920], in_=po3a[:, 0:384])
                store_cols(sig, out_sb, 1024, 1920)
                nc.vector.tensor_copy(out=out_sb[:, 1920:2048], in_=po3b[:, 0:128])
                store_cols(sig, out_sb, 1920, 2048)

            first = 0
            for g in CHUNK_UNITS:
                load_chunk(first, g)
                first += g
            for sig in range(SIGS - 1):
                compute_store(sig)
            compute_store_last(SIGS - 1)

    nc.compile()
    return nc


def _get_nc():
    if "nc" not in _NC_CACHE:
        _NC_CACHE["nc"] = _build_nc()
    return _NC_CACHE["nc"]


def _build_h4(h):
    h4 = np.zeros((K, 512), np.float32)
    qp = np.arange(64)
    for t in range(16):
        for r in range(8):
            h4[qp + t, 8 * qp + r] = FACTOR * h[(7 - r) + 8 * t]
    return h4


def _run(x_real, x_imag, fir_filter, trace=False):
    h4 = _build_h4(np.asarray(fir_filter, np.float32)).astype(np.float16)
    # host-side im2col for all 128 signals: T4[k, m] = x_pad[64m + k]
    xpad = np.zeros((2, B, NPAD), np.float16)
    xpad[0, :, 7 : 7 + N] = x_real
    xpad[1, :, 7 : 7 + N] = x_imag
    # windows[part, b, m, k] = xpad[part, b, 64m + k]
    windows = sliding_window_view(xpad, K, axis=2)[:, :, ::64, :]  # [2, B, 512, 79]
    in_maps = []
    for c in range(N_CORES):
        rows = slice(c * ROWS_PER_CORE, (c + 1) * ROWS_PER_CORE)
        # t4c[k, 512 + 512*s + m], signals = 8 real rows then 8 imag rows;
        # cols 0-511 carry H4 so chunk 0 delivers weights + sig 0 in one DMA
        t4c = np.zeros((128, 512 + SIGS * M), np.float16)
        t4c[:K, :512] = h4
        t4c[:K, 512:] = (
            windows[:, rows].reshape(SIGS, M, K).transpose(2, 0, 1).reshape(K, -1)
        )
        in_maps.append({"t4": t4c.reshape(-1)})
    nc = _get_nc()
    res = run_bass_kernel_spmd(nc, in_maps, core_ids=list(range(N_CORES)), trace=trace)
    out = np.empty((2, B, NOUT), np.float32)
    for c in range(N_CORES):
        yc = res.results[c]["y"]
        # y_dev[sig, i, 512t + n] = y[sig, 65536t + 512i + n]
        yc = yc.reshape(SIGS, 128, TILES, 512).transpose(0, 2, 1, 3).reshape(SIGS, NOUT)
        rows = slice(c * ROWS_PER_CORE, (c + 1) * ROWS_PER_CORE)
        out[0, rows] = yc[:ROWS_PER_CORE]
        out[1, rows] = yc[ROWS_PER_CORE:]
    return out, res


def kernel(x_real, x_imag, fir_filter, factor):
    assert int(factor) == FACTOR
    x_real = np.asarray(x_real, np.float32)
    x_imag = np.asarray(x_imag, np.float32)
    assert x_real.shape == (B, N) and x_imag.shape == (B, N)
    out, _ = _run(x_real, x_imag, fir_filter)
    return out


# revision 57
# speedup vs baseline: 1.1114x; 1.1114x over previous
"""Trainium2 Bass kernel for nn_Interpolator: zero-stuff upsample x8 + 128-tap FIR (SAME) + x8 gain.

Polyphase formulation: with m indexing 64-sample rows of x and n = 8*q' + r in [0, 512),
    y[512*m + n] = sum_{k=0}^{78} T4[k, m] * H4[k, n]
where T4[k, m] = x[64*m + k - 7] (zero-padded) and
    H4[k, 8*q'+r] = 8 * h[(7-r) + 8*(k-q')]  for 0 <= k-q' <= 15, else 0.

Per core (8 cores, batch-parallel): 16 signals (8 batch rows x {real, imag}).

The im2col (T4) is built on the HOST and shipped as a [128, 512 + 16*512] fp16
DRAM tensor (k-major; rows 79-127 junk), so the kernel needs NO xbar
DMA-transposes at all — that removes the serial xbar block + mode-transition
drains of the v1 kernel.  Cols 0-511 carry H4 so the first chunked load
delivers the streamed filter matrix AND sig 0's first weights in ONE DMA (one
completion receipt on the critical path instead of two).  All loads and stores
are plain HWDGE DMAs on the sync queue; stores use a PERMUTED, per-partition
contiguous DRAM layout (y_dev[sig, i, 512*t + n] = y[sig, 65536*t + 512*i + n],
AP [[2048,128],[1,2048]] = 128 x 4KB descriptors); the host un-permutes.

The PE is clock-capped at 1.2 GHz on this part (the HAM never lifts K=4/8 even
after 27 us of gapless matmuls; under thermal pressure it sags to ~1.1), so
the 64 matmuls (512 streamed cols each, 427 ns) are the 27.3 us critical path;
loads/casts/stores pipeline behind it with zero MM gaps.  Each matmul gets its
OWN single-bank [128, 512] PSUM tile (8 bufs = all 8 banks = 2 signals of
pipeline depth): per-MM casts alternate vector/scalar, and no cast ever shares
a PSUM tile with a pending matmul (Tile serializes same-tile access at tile
granularity, which would stall the PE).  Store completions (the HBM
write-receipt that fires each DMA's final sem inc) process SERIALLY at ~320 ns
per store and ~1-3 us behind the data, so the kernel-end drain is gated by the
receipt chain of the last few stores: every signal stores in TWO 256 KB halves
(the first issued mid-compute) so receipts spread out instead of bunching, and
the last signal's final matmul is split N=384+N=128 (same stationary, zero
extra PE cycles) so the very last cast/store are tiny and the drain starts as
early as possible after the last PE cycle.  y is fp16 on device; the host
casts to fp32.
"""

import numpy as np
from numpy.lib.stride_tricks import sliding_window_view

import concourse.bass as bass
import concourse.tile as tile
from concourse import bacc, mybir
from concourse.bass_utils import run_bass_kernel_spmd

B = 64
N = 32768
FACTOR = 8
NOUT = N * FACTOR  # 262144
N_CORES = 8
ROWS_PER_CORE = B // N_CORES  # 8
SIGS = 2 * ROWS_PER_CORE  # 16 signals per core (real rows then imag rows)
K = 79  # contraction window length
NPAD = 32832  # 7 leading zeros + N + 57 trailing zeros; = 64*513
M = 512  # T4 columns (m-values) per signal
TILES = 4  # out tiles per signal, each [128 m-rows, 512 samples]

# Load chunks in units of 128 T4 columns (1 signal = 4 units); the first
# chunk carries H4 (512 lead cols) + sig 0's first matmul tile so the PE can
# start as early as possible.
CHUNK_UNITS = (2, 6, 12, 20, 24)  # sums to 64 = SIGS*4

_F16 = mybir.dt.float16
_F32 = mybir.dt.float32

_NC_CACHE = {}


def _build_nc():
    nc = bacc.Bacc(
        "TRN2",
        target_bir_lowering=False,
        debug=False,
        enable_asserts=False,
        num_devices=N_CORES,
    )
    # t4x holds [128 rows, 512 + SIGS*512 cols]: cols 0-511 = H4 (rows 79+ junk),
    # cols 512.. = T4 of the 16 signals (512 m-cols each)
    t4 = nc.dram_tensor("t4", [128 * (512 + SIGS * M)], _F16, kind="ExternalInput")
    y = nc.dram_tensor("y", [SIGS, NOUT], _F16, kind="ExternalOutput")

    with tile.TileContext(nc) as tc:
        with (
            tc.tile_pool(name="t4pool", bufs=len(CHUNK_UNITS)) as t4pool,
            tc.tile_pool(name="opool", bufs=8) as opool,
            tc.tile_pool(name="po", bufs=8, space="PSUM") as po_pool,
        ):
            # per-(sig, t) map to (tile, local col base) after its chunk's load
            t4_of_tile = {}
            h4_sb = None  # set by the first chunk load (h4 rides along)
            DSTRIDE = 512 + SIGS * M  # row stride of t4x in DRAM

            def load_chunk(first_unit, n_units):
                nonlocal h4_sb
                lead = 512 if first_unit == 0 else 0  # h4 rides in chunk 0
                w = 128 * n_units + lead
                T4g = t4pool.tile([128, w], _F16, tag="t4")
                off = 512 + first_unit * 128 - lead
                nc.sync.dma_start(
                    out=T4g[:, :],
                    in_=bass.AP(tensor=t4, offset=off, ap=[[DSTRIDE, 128], [1, w]]),
                )
                if first_unit == 0:
                    h4_sb = T4g
                for u in range(n_units):
                    g = first_unit + u
                    t4_of_tile[(g // 4, g % 4)] = (T4g, 128 * u + lead)

            def store_cols(sig, out_sb, c0, c1, eng=None):
                (eng or nc.sync).dma_start(
                    out=bass.AP(
                        tensor=y,
                        offset=sig * NOUT + c0,
                        ap=[[2048, 128], [1, c1 - c0]],
                    ),
                    in_=out_sb[:, c0:c1],
                )

            def mm(sig, t, po_slice):
                T4g, base = t4_of_tile[(sig, t)]
                nc.tensor.matmul(
                    po_slice,
                    T4g[0:K, base : base + 128],
                    h4_sb[0:K, 0:512],
                    start=True,
                    stop=True,
                )

            def compute_store(sig):
                """4 matmuls, each into its own single-bank PSUM tile, per-MM
                casts alternating engines, two 256 KB half stores."""
                out_sb = opool.tile([128, TILES * 512], _F16)
                for t in range(TILES):
                    po = po_pool.tile([128, 512], _F32, tag="po")
                    mm(sig, t, po[:, :])
                    sl = slice(512 * t, 512 * (t + 1))
                    if t % 2 == 0:
                        nc.vector.tensor_copy(out=out_sb[:, sl], in_=po[:, :])
                    else:
                        nc.scalar.copy(out=out_sb[:, sl], in_=po[:, :])
                    if t == 1:
                        # store the first half early: receipts spread out
                        # instead of bunching (they serialize at ~320ns), and
                        # the final drain only waits on the last small pieces
                        store_cols(sig, out_sb, 0, 1024)
                # store: y_dev[sig, i, c] = out_sb[i, c]  (per-partition 4KB contig)
                store_cols(sig, out_sb, 1024, 2048)

            def compute_store_last(sig):
                """Last signal: the final matmul is split N=384+N=128 (same
                stationary, zero extra PE cycles) so the very last cast and
                store are tiny and the end-of-kernel drain's receipt chain
                starts as early as possible after the last PE cycle."""
                out_sb = opool.tile([128, TILES * 512], _F16)
                po0 = po_pool.tile([128, 512], _F32, tag="po")
                mm(sig, 0, po0[:, :])
                nc.vector.tensor_copy(out=out_sb[:, 0:512], in_=po0[:, :])
                po1 = po_pool.tile([128, 512], _F32, tag="po")
                mm(sig, 1, po1[:, :])
                nc.scalar.copy(out=out_sb[:, 512:1024], in_=po1[:, :])
                store_cols(sig, out_sb, 0, 1024)  # streams while t2/t3 compute
                po2 = po_pool.tile([128, 512], _F32, tag="po")
                mm(sig, 2, po2[:, :])
                nc.vector.tensor_copy(out=out_sb[:, 1024:1536], in_=po2[:, :])
                T4g, base = t4_of_tile[(sig, 3)]
                po3a = po_pool.tile([128, 512], _F32, tag="po")
                nc.tensor.matmul(
                    po3a[:, 0:384],
                    T4g[0:K, base : base + 128],
                    h4_sb[0:K, 0:384],
                    start=True,
                    stop=True,
                )
                po3b = po_pool.tile([128, 512], _F32, tag="po")
                nc.tensor.matmul(
                    po3b[:, 0:128],
                    T4g[0:K, base : base + 128],
                    h4_sb[0:K, 384:512],
                    start=True,
                    stop=True,
                )
                nc.scalar.copy(out=out_sb[:, 1536:1920], in_=po3a[:, 0:384])
                store_cols(sig, out_sb, 1024, 1920)
                nc.vector.tensor_copy(out=out_sb[:, 1920:2048], in_=po3b[:, 0:128])
                store_cols(sig, out_sb, 1920, 2048)

            first = 0
            for g in CHUNK_UNITS:
                load_chunk(first, g)
                first += g
            for sig in range(SIGS - 1):
                compute_store(sig)
            compute_store_last(SIGS - 1)

    nc.compile()
    return nc


def _get_nc():
    if "nc" not in _NC_CACHE:
        _NC_CACHE["nc"] = _build_nc()
    return _NC_CACHE["nc"]


def _build_h4(h):
    h4 = np.zeros((K, 512), np.float32)
    qp = np.arange(64)
    for t in range(16):
        for r in range(8):
            h4[qp + t, 8 * qp + r] = FACTOR * h[(7 - r) + 8 * t]
    return h4


def _run(x_real, x_imag, fir_filter, trace=False):
    h4 = _build_h4(np.asarray(fir_filter, np.float32)).astype(np.float16)
    # host-side im2col for all 128 signals: T4[k, m] = x_pad[64m + k]
    xpad = np.zeros((2, B, NPAD), np.float16)
    xpad[0, :, 7 : 7 + N] = x_real
    xpad[1, :, 7 : 7 + N] = x_imag
    # windows[part, b, m, k] = xpad[part, b, 64m + k]
    windows = sliding_window_view(xpad, K, axis=2)[:, :, ::64, :]  # [2, B, 512, 79]
    in_maps = []
    for c in range(N_CORES):
        rows = slice(c * ROWS_PER_CORE, (c + 1) * ROWS_PER_CORE)
        # t4c[k, 512 + 512*s + m], signals = 8 real rows then 8 imag rows;
        # cols 0-511 carry H4 so chunk 0 delivers weights + sig 0 in one DMA
        t4c = np.zeros((128, 512 + SIGS * M), np.float16)
        t4c[:K, :512] = h4
        t4c[:K, 512:] = (
            windows[:, rows].reshape(SIGS, M, K).transpose(2, 0, 1).reshape(K, -1)
        )
        in_maps.append({"t4": t4c.reshape(-1)})
    nc = _get_nc()
    res = run_bass_kernel_spmd(nc, in_maps, core_ids=list(range(N_CORES)), trace=trace)
    out = np.empty((2, B, NOUT), np.float32)
    for c in range(N_CORES):
        yc = res.results[c]["y"]
        # y_dev[sig, i, 512t + n] = y[sig, 65536t + 512i + n]
        yc = yc.reshape(SIGS, 128, TILES, 512).transpose(0, 2, 1, 3).reshape(SIGS, NOUT)
        rows = slice(c * ROWS_PER_CORE, (c + 1) * ROWS_PER_CORE)
        out[0, rows] = yc[:ROWS_PER_CORE]
        out[1, rows] = yc[ROWS_PER_CORE:]
    return out, res


def kernel(x_real, x_imag, fir_filter, factor):
    assert int(factor) == FACTOR
    x_real = np.asarray(x_real, np.float32)
    x_imag = np.asarray(x_imag, np.float32)
    assert x_real.shape == (B, N) and x_imag.shape == (B, N)
    out, _ = _run(x_real, x_imag, fir_filter)
    return out


# revision 58
# speedup vs baseline: 1.2087x; 1.0875x over previous
"""Trainium2 Bass kernel for nn_Interpolator: zero-stuff upsample x8 + 128-tap FIR (SAME) + x8 gain.

Polyphase formulation: with m indexing 64-sample rows of x and n = 8*q' + r in [0, 512),
    y[512*m + n] = sum_{k=0}^{78} T4[k, m] * H4[k, n]
where T4[k, m] = x[64*m + k - 7] (zero-padded) and
    H4[k, 8*q'+r] = 8 * h[(7-r) + 8*(k-q')]  for 0 <= k-q' <= 15, else 0.

Per core (8 cores, batch-parallel): 16 signals (8 batch rows x {real, imag}).

The im2col (T4) is built on the HOST and shipped as a [128, 512 + 16*512] fp16
DRAM tensor (k-major; rows 79-127 junk), so the kernel needs NO xbar
DMA-transposes at all — that removes the serial xbar block + mode-transition
drains of the v1 kernel.  Cols 0-511 carry H4 so the first chunked load
delivers the streamed filter matrix AND sig 0's first weights in ONE DMA (one
completion receipt on the critical path instead of two).  All loads and stores
are plain HWDGE DMAs on the sync queue; stores use a PERMUTED, per-partition
contiguous DRAM layout (y_dev[sig, i, 512*t + n] = y[sig, 65536*t + 512*i + n],
AP [[2048,128],[1,2048]] = 128 x 4KB descriptors); the host un-permutes.

The PE is clock-capped at 1.2 GHz on this part (the HAM never lifts K=4/8 even
after 27 us of gapless matmuls; under thermal pressure it sags to ~1.1), so
the 64 matmuls (512 streamed cols each, 427 ns) are the 27.3 us critical path;
loads/casts/stores pipeline behind it with zero MM gaps.  Each matmul gets its
OWN single-bank [128, 512] PSUM tile (8 bufs = all 8 banks = 2 signals of
pipeline depth): per-MM casts alternate vector/scalar, and no cast ever shares
a PSUM tile with a pending matmul (Tile serializes same-tile access at tile
granularity, which would stall the PE).  Store completions (the HBM
write-receipt that fires each DMA's final sem inc) process SERIALLY at ~320 ns
per store and ~1-3 us behind the data, so the kernel-end drain is gated by the
receipt chain of the last few stores: every signal stores in TWO 256 KB halves
(the first issued mid-compute) so receipts spread out instead of bunching, and
the last signal's final matmul is split N=384+N=128 (same stationary, zero
extra PE cycles) so the very last cast/store are tiny and the drain starts as
early as possible after the last PE cycle.  y is fp16 on device; the host
casts to fp32.
"""

import numpy as np
from numpy.lib.stride_tricks import sliding_window_view

import concourse.bass as bass
import concourse.tile as tile
from concourse import bacc, mybir
from concourse.bass_utils import run_bass_kernel_spmd

B = 64
N = 32768
FACTOR = 8
NOUT = N * FACTOR  # 262144
N_CORES = 8
ROWS_PER_CORE = B // N_CORES  # 8
SIGS = 2 * ROWS_PER_CORE  # 16 signals per core (real rows then imag rows)
K = 79  # contraction window length
NPAD = 32832  # 7 leading zeros + N + 57 trailing zeros; = 64*513
M = 512  # T4 columns (m-values) per signal
TILES = 4  # out tiles per signal, each [128 m-rows, 512 samples]

# Load chunks in units of 128 T4 columns (1 signal = 4 units); the first
# chunk carries H4 (512 lead cols) + sig 0's first matmul tile so the PE can
# start as early as possible.
CHUNK_UNITS = (2, 6, 12, 20, 24)  # sums to 64 = SIGS*4

_F16 = mybir.dt.float16
_F32 = mybir.dt.float32

_NC_CACHE = {}


def _build_nc():
    nc = bacc.Bacc(
        "TRN2",
        target_bir_lowering=False,
        debug=False,
        enable_asserts=False,
        num_devices=N_CORES,
    )
    # t4x holds [80 rows, 512 + SIGS*512 cols]: cols 0-511 = H4 (row 79 zero pad),
    # cols 512.. = T4 of the 16 signals (512 m-cols each)
    t4 = nc.dram_tensor("t4", [80 * (512 + SIGS * M)], _F16, kind="ExternalInput")
    y = nc.dram_tensor("y", [SIGS, NOUT], _F16, kind="ExternalOutput")

    with tile.TileContext(nc) as tc:
        with (
            tc.tile_pool(name="t4pool", bufs=len(CHUNK_UNITS)) as t4pool,
            tc.tile_pool(name="opool", bufs=8) as opool,
            tc.tile_pool(name="po", bufs=8, space="PSUM") as po_pool,
        ):
            # per-(sig, t) map to (tile, local col base) after its chunk's load
            t4_of_tile = {}
            h4_sb = None  # set by the first chunk load (h4 rides along)
            DSTRIDE = 512 + SIGS * M  # row stride of t4x in DRAM

            def load_chunk(first_unit, n_units):
                nonlocal h4_sb
                lead = 512 if first_unit == 0 else 0  # h4 rides in chunk 0
                w = 128 * n_units + lead
                T4g = t4pool.tile([80, w], _F16, tag="t4")
                off = 512 + first_unit * 128 - lead
                nc.sync.dma_start(
                    out=T4g[:, :],
                    in_=bass.AP(tensor=t4, offset=off, ap=[[DSTRIDE, 80], [1, w]]),
                )
                if first_unit == 0:
                    h4_sb = T4g
                for u in range(n_units):
                    g = first_unit + u
                    t4_of_tile[(g // 4, g % 4)] = (T4g, 128 * u + lead)

            def store_cols(sig, out_sb, c0, c1, eng=None):
                (eng or nc.sync).dma_start(
                    out=bass.AP(
                        tensor=y,
                        offset=sig * NOUT + c0,
                        ap=[[2048, 128], [1, c1 - c0]],
                    ),
                    in_=out_sb[:, c0:c1],
                )

            def mm(sig, t, po_slice):
                T4g, base = t4_of_tile[(sig, t)]
                nc.tensor.matmul(
                    po_slice,
                    T4g[0:K, base : base + 128],
                    h4_sb[0:K, 0:512],
                    start=True,
                    stop=True,
                )

            def compute_store(sig):
                """4 matmuls, each into its own single-bank PSUM tile, per-MM
                casts alternating engines, two 256 KB half stores."""
                out_sb = opool.tile([128, TILES * 512], _F16)
                for t in range(TILES):
                    po = po_pool.tile([128, 512], _F32, tag="po")
                    mm(sig, t, po[:, :])
                    sl = slice(512 * t, 512 * (t + 1))
                    if t % 2 == 0:
                        nc.vector.tensor_copy(out=out_sb[:, sl], in_=po[:, :])
                    else:
                        nc.scalar.copy(out=out_sb[:, sl], in_=po[:, :])
                    if t == 1:
                        # store the first half early: receipts spread out
                        # instead of bunching (they serialize at ~320ns), and
                        # the final drain only waits on the last small pieces
                        store_cols(sig, out_sb, 0, 1024)
                # store: y_dev[sig, i, c] = out_sb[i, c]  (per-partition 4KB contig)
                store_cols(sig, out_sb, 1024, 2048)

            def compute_store_last(sig):
                """Last signal: the final matmul is split N=384+N=128 (same
                stationary, zero extra PE cycles) so the very last cast and
                store are tiny and the end-of-kernel drain's receipt chain
                starts as early as possible after the last PE cycle."""
                out_sb = opool.tile([128, TILES * 512], _F16)
                po0 = po_pool.tile([128, 512], _F32, tag="po")
                mm(sig, 0, po0[:, :])
                nc.vector.tensor_copy(out=out_sb[:, 0:512], in_=po0[:, :])
                po1 = po_pool.tile([128, 512], _F32, tag="po")
                mm(sig, 1, po1[:, :])
                nc.scalar.copy(out=out_sb[:, 512:1024], in_=po1[:, :])
                store_cols(sig, out_sb, 0, 1024)  # streams while t2/t3 compute
                po2 = po_pool.tile([128, 512], _F32, tag="po")
                mm(sig, 2, po2[:, :])
                nc.vector.tensor_copy(out=out_sb[:, 1024:1536], in_=po2[:, :])
                T4g, base = t4_of_tile[(sig, 3)]
                po3a = po_pool.tile([128, 512], _F32, tag="po")
                nc.tensor.matmul(
                    po3a[:, 0:384],
                    T4g[0:K, base : base + 128],
                    h4_sb[0:K, 0:384],
                    start=True,
                    stop=True,
                )
                po3b = po_pool.tile([128, 512], _F32, tag="po")
                nc.tensor.matmul(
                    po3b[:, 0:128],
                    T4g[0:K, base : base + 128],
                    h4_sb[0:K, 384:512],
                    start=True,
                    stop=True,
                )
                nc.scalar.copy(out=out_sb[:, 1536:1920], in_=po3a[:, 0:384])
                store_cols(sig, out_sb, 1024, 1920)
                nc.vector.tensor_copy(out=out_sb[:, 1920:2048], in_=po3b[:, 0:128])
                store_cols(sig, out_sb, 1920, 2048)

            first = 0
            for g in CHUNK_UNITS:
                load_chunk(first, g)
                first += g
            for sig in range(SIGS - 1):
                compute_store(sig)
            compute_store_last(SIGS - 1)

    nc.compile()
    return nc


def _get_nc():
    if "nc" not in _NC_CACHE:
        _NC_CACHE["nc"] = _build_nc()
    return _NC_CACHE["nc"]


def _build_h4(h):
    h4 = np.zeros((K, 512), np.float32)
    qp = np.arange(64)
    for t in range(16):
        for r in range(8):
            h4[qp + t, 8 * qp + r] = FACTOR * h[(7 - r) + 8 * t]
    return h4


def _run(x_real, x_imag, fir_filter, trace=False):
    h4 = _build_h4(np.asarray(fir_filter, np.float32)).astype(np.float16)
    # host-side im2col for all 128 signals: T4[k, m] = x_pad[64m + k]
    xpad = np.zeros((2, B, NPAD), np.float16)
    xpad[0, :, 7 : 7 + N] = x_real
    xpad[1, :, 7 : 7 + N] = x_imag
    # windows[part, b, m, k] = xpad[part, b, 64m + k]
    windows = sliding_window_view(xpad, K, axis=2)[:, :, ::64, :]  # [2, B, 512, 79]
    in_maps = []
    for c in range(N_CORES):
        rows = slice(c * ROWS_PER_CORE, (c + 1) * ROWS_PER_CORE)
        # t4c[k, 512 + 512*s + m], signals = 8 real rows then 8 imag rows;
        # cols 0-511 carry H4 so chunk 0 delivers weights + sig 0 in one DMA
        t4c = np.zeros((80, 512 + SIGS * M), np.float16)
        t4c[:K, :512] = h4
        t4c[:K, 512:] = (
            windows[:, rows].reshape(SIGS, M, K).transpose(2, 0, 1).reshape(K, -1)
        )
        in_maps.append({"t4": t4c.reshape(-1)})
    nc = _get_nc()
    res = run_bass_kernel_spmd(nc, in_maps, core_ids=list(range(N_CORES)), trace=trace)
    out = np.empty((2, B, NOUT), np.float32)
    for c in range(N_CORES):
        yc = res.results[c]["y"]
        # y_dev[sig, i, 512t + n] = y[sig, 65536t + 512i + n]
        yc = yc.reshape(SIGS, 128, TILES, 512).transpose(0, 2, 1, 3).reshape(SIGS, NOUT)
        rows = slice(c * ROWS_PER_CORE, (c + 1) * ROWS_PER_CORE)
        out[0, rows] = yc[:ROWS_PER_CORE]
        out[1, rows] = yc[ROWS_PER_CORE:]
    return out, res


def kernel(x_real, x_imag, fir_filter, factor):
    assert int(factor) == FACTOR
    x_real = np.asarray(x_real, np.float32)
    x_imag = np.asarray(x_imag, np.float32)
    assert x_real.shape == (B, N) and x_imag.shape == (B, N)
    out, _ = _run(x_real, x_imag, fir_filter)
    return out


# revision 59
# speedup vs baseline: 1.2211x; 1.0103x over previous
"""Trainium2 Bass kernel for nn_Interpolator: zero-stuff upsample x8 + 128-tap FIR (SAME) + x8 gain.

Polyphase formulation: with m indexing 64-sample rows of x and n = 8*q' + r in [0, 512),
    y[512*m + n] = sum_{k=0}^{78} T4[k, m] * H4[k, n]
where T4[k, m] = x[64*m + k - 7] (zero-padded) and
    H4[k, 8*q'+r] = 8 * h[(7-r) + 8*(k-q')]  for 0 <= k-q' <= 15, else 0.

Per core (8 cores, batch-parallel): 16 signals (8 batch rows x {real, imag}).

The im2col (T4) is built on the HOST and shipped as a dense [80, 512 + 16*512]
fp16 DRAM tensor (k-major; row 79 zero pad so the partition count stays on the
8-per-SDMA-engine granularity), so the kernel needs NO xbar DMA-transposes at
all — that removes the serial xbar block + mode-transition drains of the v1
kernel — and no junk rows are ever transferred (38% less load traffic than a
128-row layout; same transfer time per byte since even/odd SBUF ports stay
balanced, but fewer descriptors and less HBM contention with the stores).  Cols 0-511 carry H4 so the first chunked load
delivers the streamed filter matrix AND sig 0's first weights in ONE DMA (one
completion receipt on the critical path instead of two).  All loads and stores
are plain HWDGE DMAs on the sync queue; stores use a PERMUTED, per-partition
contiguous DRAM layout (y_dev[sig, i, 512*t + n] = y[sig, 65536*t + 512*i + n],
AP [[2048,128],[1,2048]] = 128 x 4KB descriptors); the host un-permutes.

The PE is clock-capped at 1.2 GHz on this part (the HAM never lifts K=4/8 even
after 27 us of gapless matmuls; under thermal pressure it sags to ~1.1), so
the 64 matmuls (512 streamed cols each, 427 ns) are the 27.3 us critical path;
loads/casts/stores pipeline behind it with zero MM gaps.  Each matmul gets its
OWN single-bank [128, 512] PSUM tile (8 bufs = all 8 banks = 2 signals of
pipeline depth): per-MM casts alternate vector/scalar, and no cast ever shares
a PSUM tile with a pending matmul (Tile serializes same-tile access at tile
granularity, which would stall the PE).  Store completions (the HBM
write-receipt that fires each DMA's final sem inc) process SERIALLY at ~320 ns
per store and ~1-3 us behind the data, so the kernel-end drain is gated by the
receipt chain of the last few stores: every signal stores in TWO 256 KB halves
(the first issued mid-compute) so receipts spread out instead of bunching, and
the last signal's final matmul is split N=384+N=128 (same stationary, zero
extra PE cycles) so the very last cast/store are tiny and the drain starts as
early as possible after the last PE cycle.  y is fp16 on device; the host
casts to fp32.
"""

import numpy as np
from numpy.lib.stride_tricks import sliding_window_view

import concourse.bass as bass
import concourse.tile as tile
from concourse import bacc, mybir
from concourse.bass_utils import run_bass_kernel_spmd

B = 64
N = 32768
FACTOR = 8
NOUT = N * FACTOR  # 262144
N_CORES = 8
ROWS_PER_CORE = B // N_CORES  # 8
SIGS = 2 * ROWS_PER_CORE  # 16 signals per core (real rows then imag rows)
K = 79  # contraction window length
NPAD = 32832  # 7 leading zeros + N + 57 trailing zeros; = 64*513
M = 512  # T4 columns (m-values) per signal
TILES = 4  # out tiles per signal, each [128 m-rows, 512 samples]

# Load chunks in units of 128 T4 columns (1 signal = 4 units); the first
# chunk carries H4 (512 lead cols) + sig 0's first matmul tile so the PE can
# start as early as possible.
CHUNK_UNITS = (2, 6, 12, 20, 24)  # sums to 64 = SIGS*4

_F16 = mybir.dt.float16
_F32 = mybir.dt.float32

_NC_CACHE = {}


def _build_nc():
    nc = bacc.Bacc(
        "TRN2",
        target_bir_lowering=False,
        debug=False,
        enable_asserts=False,
        num_devices=N_CORES,
    )
    # t4x holds [80 rows, 512 + SIGS*512 cols]: cols 0-511 = H4 (row 79 zero pad),
    # cols 512.. = T4 of the 16 signals (512 m-cols each)
    t4 = nc.dram_tensor("t4", [80 * (512 + SIGS * M)], _F16, kind="ExternalInput")
    y = nc.dram_tensor("y", [SIGS, NOUT], _F16, kind="ExternalOutput")

    with tile.TileContext(nc) as tc:
        with (
            tc.tile_pool(name="t4pool", bufs=len(CHUNK_UNITS)) as t4pool,
            tc.tile_pool(name="opool", bufs=8) as opool,
            tc.tile_pool(name="po", bufs=8, space="PSUM") as po_pool,
        ):
            # per-(sig, t) map to (tile, local col base) after its chunk's load
            t4_of_tile = {}
            h4_sb = None  # set by the first chunk load (h4 rides along)
            DSTRIDE = 512 + SIGS * M  # row stride of t4x in DRAM

            def load_chunk(first_unit, n_units):
                nonlocal h4_sb
                lead = 512 if first_unit == 0 else 0  # h4 rides in chunk 0
                w = 128 * n_units + lead
                T4g = t4pool.tile([80, w], _F16, tag="t4")
                off = 512 + first_unit * 128 - lead
                nc.sync.dma_start(
                    out=T4g[:, :],
                    in_=bass.AP(tensor=t4, offset=off, ap=[[DSTRIDE, 80], [1, w]]),
                )
                if first_unit == 0:
                    h4_sb = T4g
                for u in range(n_units):
                    g = first_unit + u
                    t4_of_tile[(g // 4, g % 4)] = (T4g, 128 * u + lead)

            def store_cols(sig, out_sb, c0, c1, eng=None):
                (eng or nc.sync).dma_start(
                    out=bass.AP(
                        tensor=y,
                        offset=sig * NOUT + c0,
                        ap=[[2048, 128], [1, c1 - c0]],
                    ),
                    in_=out_sb[:, c0:c1],
                )

            def mm(sig, t, po_slice):
                T4g, base = t4_of_tile[(sig, t)]
                nc.tensor.matmul(
                    po_slice,
                    T4g[0:K, base : base + 128],
                    h4_sb[0:K, 0:512],
                    start=True,
                    stop=True,
                )

            def compute_store(sig):
                """4 matmuls, each into its own single-bank PSUM tile, per-MM
                casts alternating engines, two 256 KB half stores."""
                out_sb = opool.tile([128, TILES * 512], _F16)
                for t in range(TILES):
                    po = po_pool.tile([128, 512], _F32, tag="po")
                    mm(sig, t, po[:, :])
                    sl = slice(512 * t, 512 * (t + 1))
                    if t % 2 == 0:
                        nc.vector.tensor_copy(out=out_sb[:, sl], in_=po[:, :])
                    else:
                        nc.scalar.copy(out=out_sb[:, sl], in_=po[:, :])
                    if t == 1:
                        # store the first half early: receipts spread out
                        # instead of bunching (they serialize at ~320ns), and
                        # the final drain only waits on the last small pieces
                        store_cols(sig, out_sb, 0, 1024)
                # store: y_dev[sig, i, c] = out_sb[i, c]  (per-partition 4KB contig)
                store_cols(sig, out_sb, 1024, 2048)

            def compute_store_last(sig):
                """Last signal: the final matmul is split N=384+N=128 (same
                stationary, zero extra PE cycles) so the very last cast and
                store are tiny and the end-of-kernel drain's receipt chain
                starts as early as possible after the last PE cycle."""
                out_sb = opool.tile([128, TILES * 512], _F16)
                po0 = po_pool.tile([128, 512], _F32, tag="po")
                mm(sig, 0, po0[:, :])
                nc.vector.tensor_copy(out=out_sb[:, 0:512], in_=po0[:, :])
                po1 = po_pool.tile([128, 512], _F32, tag="po")
                mm(sig, 1, po1[:, :])
                nc.scalar.copy(out=out_sb[:, 512:1024], in_=po1[:, :])
                store_cols(sig, out_sb, 0, 1024)  # streams while t2/t3 compute
                po2 = po_pool.tile([128, 512], _F32, tag="po")
                mm(sig, 2, po2[:, :])
                nc.vector.tensor_copy(out=out_sb[:, 1024:1536], in_=po2[:, :])
                T4g, base = t4_of_tile[(sig, 3)]
                po3a = po_pool.tile([128, 512], _F32, tag="po")
                nc.tensor.matmul(
                    po3a[:, 0:384],
                    T4g[0:K, base : base + 128],
                    h4_sb[0:K, 0:384],
                    start=True,
                    stop=True,
                )
                po3b = po_pool.tile([128, 512], _F32, tag="po")
                nc.tensor.matmul(
                    po3b[:, 0:128],
                    T4g[0:K, base : base + 128],
                    h4_sb[0:K, 384:512],
                    start=True,
                    stop=True,
                )
                nc.scalar.copy(out=out_sb[:, 1536:1920], in_=po3a[:, 0:384])
                store_cols(sig, out_sb, 1024, 1920)
                nc.vector.tensor_copy(out=out_sb[:, 1920:2048], in_=po3b[:, 0:128])
                store_cols(sig, out_sb, 1920, 2048)

            first = 0
            for g in CHUNK_UNITS:
                load_chunk(first, g)
                first += g
            for sig in range(SIGS - 1):
                compute_store(sig)
            compute_store_last(SIGS - 1)

    nc.compile()
    return nc


def _get_nc():
    if "nc" not in _NC_CACHE:
        _NC_CACHE["nc"] = _build_nc()
    return _NC_CACHE["nc"]


def _build_h4(h):
    h4 = np.zeros((K, 512), np.float32)
    qp = np.arange(64)
    for t in range(16):
        for r in range(8):
            h4[qp + t, 8 * qp + r] = FACTOR * h[(7 - r) + 8 * t]
    return h4


def _run(x_real, x_imag, fir_filter, trace=False):
    h4 = _build_h4(np.asarray(fir_filter, np.float32)).astype(np.float16)
    # host-side im2col for all 128 signals: T4[k, m] = x_pad[64m + k]
    xpad = np.zeros((2, B, NPAD), np.float16)
    xpad[0, :, 7 : 7 + N] = x_real
    xpad[1, :, 7 : 7 + N] = x_imag
    # windows[part, b, m, k] = xpad[part, b, 64m + k]
    windows = sliding_window_view(xpad, K, axis=2)[:, :, ::64, :]  # [2, B, 512, 79]
    in_maps = []
    for c in range(N_CORES):
        rows = slice(c * ROWS_PER_CORE, (c + 1) * ROWS_PER_CORE)
        # t4c[k, 512 + 512*s + m], signals = 8 real rows then 8 imag rows;
        # cols 0-511 carry H4 so chunk 0 delivers weights + sig 0 in one DMA
        t4c = np.zeros((80, 512 + SIGS * M), np.float16)
        t4c[:K, :512] = h4
        t4c[:K, 512:] = (
            windows[:, rows].reshape(SIGS, M, K).transpose(2, 0, 1).reshape(K, -1)
        )
        in_maps.append({"t4": t4c.reshape(-1)})
    nc = _get_nc()
    res = run_bass_kernel_spmd(nc, in_maps, core_ids=list(range(N_CORES)), trace=trace)
    out = np.empty((2, B, NOUT), np.float32)
    for c in range(N_CORES):
        yc = res.results[c]["y"]
        # y_dev[sig, i, 512t + n] = y[sig, 65536t + 512i + n]
        yc = yc.reshape(SIGS, 128, TILES, 512).transpose(0, 2, 1, 3).reshape(SIGS, NOUT)
        rows = slice(c * ROWS_PER_CORE, (c + 1) * ROWS_PER_CORE)
        out[0, rows] = yc[:ROWS_PER_CORE]
        out[1, rows] = yc[ROWS_PER_CORE:]
    return out, res


def kernel(x_real, x_imag, fir_filter, factor):
    assert int(factor) == FACTOR
    x_real = np.asarray(x_real, np.float32)
    x_imag = np.asarray(x_imag, np.float32)
    assert x_real.shape == (B, N) and x_imag.shape == (B, N)
    out, _ = _run(x_real, x_imag, fir_filter)
    return out
